# revision 1
# baseline (speedup 1.0000x reference)
"""AdaptiveNodeSampler TRN2 kernel — single-stage, exact f32, 3-engine.

Per core (128 rows on SBUF partitions, full N=2048 per row):
  - candidates stream in contiguous f32 chunks [P, 128, 128] (~400 GB/s)
  - scores S[r,n] = sum_d qt[r,d]*C[r,n,d] split across engines:
      TensorE:  d < DPE   via  S += diag(qt[:,d]) @ C[:,:,d]  (PSUM chain)
      VectorE:  d >= DPE  via  broadcast-mult + segmented reduce
  - softmax stats + phase = ln(exp(S-m)+beta) - t2(u)   (ScalarE)
  - top-32 indices via 4x (max8 -> max_index -> match_replace)

bk cancels in softmax (per-row constant); scale folded into qt.
"""

import os
import sys

sys.path.insert(0, "/opt/trn_rl_repo")

import numpy as np

P = 128
N = 2048
D = 128
K = 32
NCH = 128            # n-chunk size
DPE = int(os.environ.get("ANS_DPE", "52"))   # d's on TensorE
NCORES = 8
GAMMA = 0.1
EPS = 1e-10
NEG_INF = -1.0e30

_CACHE = {}
LAST_RESULT = None


def _build():
    import concourse.bass as bass
    import concourse.bacc as bacc
    import concourse.tile as tile
    from concourse import mybir
    from concourse.masks import make_identity

    f32 = mybir.dt.float32
    i32 = mybir.dt.int32
    u32 = mybir.dt.uint32
    alu = mybir.AluOpType
    act = mybir.ActivationFunctionType
    AP = bass.AP

    def bcast_mid(ap, n):
        """[P, F] -> [P, n, F] with stride-0 middle dim."""
        return AP(tensor=ap.tensor, offset=ap.offset,
                  ap=[ap.ap[0], [0, n], ap.ap[1]])

    DDV = D - DPE

    nc = bacc.Bacc("TRN2", target_bir_lowering=False, debug=False,
                   num_devices=NCORES)

    tgt = nc.declare_dram_parameter("target", [P, D], f32, isOutput=False)
    cand = nc.declare_dram_parameter("cand", [P, N, D], f32, isOutput=False)
    u = nc.declare_dram_parameter("u", [P, N], f32, isOutput=False)
    wq = nc.declare_dram_parameter("Wq", [D, D], f32, isOutput=False)
    wk = nc.declare_dram_parameter("Wk", [D, D], f32, isOutput=False)
    bq = nc.declare_dram_parameter("bq", [D, 1], f32, isOutput=False)
    out = nc.declare_dram_parameter("out", [P, K], i32, isOutput=True)

    with tile.TileContext(nc) as tc:
        with (
            tc.tile_pool(name="consts", bufs=1) as consts,
            tc.tile_pool(name="small", bufs=1) as small,
            tc.tile_pool(name="gum", bufs=1) as gum,
            tc.tile_pool(name="spool", bufs=1) as spool,
            tc.tile_pool(name="psum_s", bufs=1, space="PSUM") as psum_s,
            tc.tile_pool(name="psum_a", bufs=2, space="PSUM") as psum_a,
        ):
            ident = consts.tile([P, P], f32)
            make_identity(nc, ident)

            wq_sb = consts.tile([D, D], f32)   # [e, f]
            nc.sync.dma_start(out=wq_sb, in_=wq[:, :])
            wk_sb = consts.tile([D, D], f32)   # [e, d]
            nc.sync.dma_start(out=wk_sb, in_=wk[:, :])
            tgt_sb = consts.tile([P, D], f32)  # [r, f]
            nc.sync.dma_start(out=tgt_sb, in_=tgt[:, :])
            bq_sb = consts.tile([D, 1], f32)
            nc.sync.dma_start(out=bq_sb, in_=bq[:, :])

            # Qt = ((target @ Wq.T + bq) @ Wk) / sqrt(D),  layout [r, d]
            tgtT_ps = psum_s.tile([D, P], f32)
            nc.tensor.transpose(tgtT_ps, tgt_sb, ident)   # [f, r]
            tgtT_sb = consts.tile([D, P], f32)
            nc.scalar.copy(tgtT_sb, tgtT_ps)

            wqT_ps = psum_s.tile([D, D], f32)
            nc.tensor.transpose(wqT_ps, wq_sb, ident)     # [f, e]
            wqT_sb = consts.tile([D, D], f32)
            nc.scalar.copy(wqT_sb, wqT_ps)

            qT_ps = psum_s.tile([D, P], f32)              # Q.T = [e, r]
            nc.tensor.matmul(qT_ps, wqT_sb, tgtT_sb, start=True, stop=True)
            qT_sb = consts.tile([D, P], f32)
            nc.vector.tensor_scalar_add(qT_sb, qT_ps, bq_sb)

            qt_ps = psum_s.tile([P, D], f32)              # Qt = [r, d]
            nc.tensor.matmul(qt_ps, qT_sb, wk_sb, start=True, stop=True)
            qt_sb = consts.tile([P, D], f32)
            nc.vector.tensor_scalar_mul(qt_sb, qt_ps,
                                        float(1.0 / np.sqrt(np.float32(D))))

            # f32 diagonal weights diag(qt[:, d]) for the PE chain
            diags = consts.tile([P, DPE, P], f32)
            for d in range(DPE):
                nc.vector.tensor_scalar_mul(diags[:, d, :], ident,
                                            qt_sb[:, d:d + 1])

            eps_sb = small.tile([P, 1], f32)
            nc.vector.memset(eps_sb, EPS)

            # ---- main loop: stream candidates, engine-split dot products
            S = spool.tile([P, N], f32)
            with tc.tile_pool(name="cpool", bufs=2) as cpool, \
                 tc.tile_pool(name="ppool", bufs=1) as ppool:
                for g in range(N // NCH):
                    ns = slice(g * NCH, (g + 1) * NCH)
                    ct = cpool.tile([P, NCH, D], f32, tag="c")
                    # split across the two HWDGE rings (SP + ACT).  ACT is
                    # kept free of compute during the main loop so its ring
                    # issues promptly; a single ring caps 8-core DMA at half
                    # rate.
                    h = NCH // 2
                    nc.sync.dma_start(out=ct[:, :h, :],
                                      in_=cand[:, g * NCH:g * NCH + h, :])
                    nc.scalar.dma_start(out=ct[:, h:, :],
                                        in_=cand[:, g * NCH + h:(g + 1) * NCH, :])
                    ps = psum_a.tile([P, NCH], f32, tag="ps")
                    for d in range(DPE):
                        nc.tensor.matmul(ps, diags[:, d, :], ct[:, :, d],
                                         start=(d == 0), stop=(d == DPE - 1))
                    nc.vector.tensor_copy(S[:, ns], ps)
                    # VectorE takes the remaining d's in four slices
                    qtr = (DDV + 3) // 4
                    bounds = list(range(DPE, D, qtr)) + [D]
                    s2 = ppool.tile([P, NCH], f32, tag="s2")
                    for (d0, d1) in zip(bounds[:-1], bounds[1:]):
                        w = d1 - d0
                        prod = ppool.tile([P, NCH, w], f32, tag="pa")
                        nc.vector.tensor_tensor(
                            out=prod, in0=ct[:, :, d0:d1],
                            in1=bcast_mid(qt_sb[:, d0:d1], NCH),
                            op=alu.mult)
                        nc.vector.tensor_reduce(
                            out=s2, in_=prod,
                            axis=mybir.AxisListType.X, op=alu.add)
                        nc.vector.tensor_add(S[:, ns], S[:, ns], s2)

            # ---- tail: phase = ln(exp(S-m)+beta) - t2, then top-32
            u_sb = gum.tile([P, N], f32, tag="g0")
            nc.sync.dma_start(out=u_sb, in_=u[:, :])
            t1 = gum.tile([P, N], f32, tag="g1")
            nc.scalar.activation(t1, u_sb, act.Ln, bias=eps_sb, scale=1.0)
            t2 = gum.tile([P, N], f32, tag="g2")
            nc.scalar.activation(t2, t1, act.Ln, bias=eps_sb, scale=-1.0)
            negm = small.tile([P, 1], f32)
            nc.vector.tensor_reduce(out=negm, in_=S,
                                    axis=mybir.AxisListType.X,
                                    op=alu.max, negate=True)
            E = gum.tile([P, N], f32, tag="g0")      # reuses u slot
            sumE = small.tile([P, 1], f32)
            nc.scalar.activation(E, S, act.Exp, bias=negm, scale=1.0,
                                 accum_out=sumE)
            beta = small.tile([P, 1], f32)
            nc.vector.tensor_scalar_mul(
                beta, sumE, float(GAMMA / ((1.0 - GAMMA) * N)))
            val = gum.tile([P, N], f32, tag="g1")    # reuses t1 slot
            nc.scalar.activation(val, E, act.Ln, bias=beta, scale=1.0)
            phase = t2                     # in-place: phase = val - t2
            nc.vector.tensor_sub(phase, val, t2)

            v8 = small.tile([P, 8], f32)
            idx = small.tile([P, K], u32)
            for r in range(K // 8):
                nc.vector.max(out=v8, in_=phase)
                nc.vector.max_index(out=idx[:, r * 8:(r + 1) * 8],
                                    in_max=v8, in_values=phase)
                if r < K // 8 - 1:
                    nc.vector.match_replace(out=phase, in_to_replace=v8,
                                            in_values=phase,
                                            imm_value=NEG_INF)

            nc.sync.dma_start(out=out[:, :],
                              in_=idx[:, :].bitcast(i32))

    nc.compile()
    return nc


def _get_nc():
    if "nc" not in _CACHE:
        _CACHE["nc"] = _build()
    return _CACHE["nc"]


def kernel(target_embed, candidate_embeds, Wq, bq, Wk, bk=None, u=None,
           num_neighbors=32, **_unused):
    global LAST_RESULT
    from concourse.bass_utils import run_bass_kernel_spmd

    assert int(num_neighbors) == K

    target = np.ascontiguousarray(np.asarray(target_embed, dtype=np.float32))
    cand = np.ascontiguousarray(np.asarray(candidate_embeds, dtype=np.float32))
    uu = np.ascontiguousarray(np.asarray(u, dtype=np.float32))
    wq_ = np.ascontiguousarray(np.asarray(Wq, dtype=np.float32))
    wk_ = np.ascontiguousarray(np.asarray(Wk, dtype=np.float32))
    bq_ = np.ascontiguousarray(np.asarray(bq, dtype=np.float32).reshape(D, 1))

    B = target.shape[0]
    assert B == P * NCORES and cand.shape == (B, N, D)

    in_maps = []
    for c in range(NCORES):
        rs = slice(c * P, (c + 1) * P)
        in_maps.append({
            "target": target[rs],
            "cand": cand[rs],
            "u": uu[rs],
            "Wq": wq_,
            "Wk": wk_,
            "bq": bq_,
        })

    nc = _get_nc()
    res = run_bass_kernel_spmd(nc, in_maps, core_ids=list(range(NCORES)))
    LAST_RESULT = res
    out = np.concatenate([res.results[c]["out"] for c in range(NCORES)],
                         axis=0)
    return out.astype(np.int32)



# revision 6
# speedup vs baseline: 1.0522x; 1.0522x over previous
"""AdaptiveNodeSampler TRN2 kernel — exact f32, fully-overlapped 3-engine.

Per core (128 rows on SBUF partitions, full N=2048 per row):
  - candidates stream in contiguous f32 chunks [P, 128, 128] across the
    two HWDGE rings (SP + ACT), double buffered.
  - scores S[r,n] = sum_d qt[r,d]*C[r,n,d] split across engines:
      TensorE:  d < DPE   via  S += diag(qt[:,d]) @ C[:,:,d]  (PSUM chain)
      VectorE:  d >= DPE  via  2x (broadcast-mult + segmented reduce)
    PE and DVE are decoupled: the PSUM combine for chunk g happens at the
    START of DVE's block g+1 (deferred-by-one), psum bufs=4, so neither
    engine ever waits on the other inside a chunk window.
  - gumbel t1/t2 precomputed on ScalarE during the loop (u loaded early);
    Exp act-table preloaded via a dummy op mid-loop.
  - per-chunk running max partials m16 -> tail reduce (negm).
  - tail: EXP(S-m, accum sumE) -> beta -> val=Ln(E+beta) (written into S)
    -> phase=val-t2 -> top-32 via 4x (max8 -> max_index -> match_replace).

bk cancels in softmax (per-row constant); scale folded into qt.
"""

import os
import sys

sys.path.insert(0, "/opt/trn_rl_repo")

import numpy as np

P = 128
N = 2048
D = 128
K = 32
NCH = 128            # n-chunk size
DPE = int(os.environ.get("ANS_DPE", "48"))   # d's on TensorE
NCORES = 8
GAMMA = 0.1
EPS = 1e-10
NEG_INF = -1.0e30

_CACHE = {}
LAST_RESULT = None


def _build():
    import concourse.bass as bass
    import concourse.bacc as bacc
    import concourse.tile as tile
    from concourse import mybir
    from concourse.masks import make_identity

    f32 = mybir.dt.float32
    i32 = mybir.dt.int32
    u32 = mybir.dt.uint32
    alu = mybir.AluOpType
    act = mybir.ActivationFunctionType
    AP = bass.AP

    def bcast_mid(ap, n):
        """[P, F] -> [P, n, F] with stride-0 middle dim."""
        return AP(tensor=ap.tensor, offset=ap.offset,
                  ap=[ap.ap[0], [0, n], ap.ap[1]])

    DDV = D - DPE
    W1 = DDV // 2          # first DVE slice width
    W2 = DDV - W1
    NG = N // NCH          # number of chunks

    nc = bacc.Bacc("TRN2", target_bir_lowering=False, debug=False,
                   num_devices=NCORES)

    tgt = nc.declare_dram_parameter("target", [P, D], f32, isOutput=False)
    cand = nc.declare_dram_parameter("cand", [P, N, D], f32, isOutput=False)
    u = nc.declare_dram_parameter("u", [P, N], f32, isOutput=False)
    wq = nc.declare_dram_parameter("Wq", [D, D], f32, isOutput=False)
    wk = nc.declare_dram_parameter("Wk", [D, D], f32, isOutput=False)
    bq = nc.declare_dram_parameter("bq", [D, 1], f32, isOutput=False)
    out = nc.declare_dram_parameter("out", [P, K], i32, isOutput=True)

    with tile.TileContext(nc) as tc:
        with (
            tc.tile_pool(name="consts", bufs=1) as consts,
            tc.tile_pool(name="small", bufs=1) as small,
            tc.tile_pool(name="gum", bufs=1) as gum,
            tc.tile_pool(name="spool", bufs=1) as spool,
            tc.tile_pool(name="cpool", bufs=2) as cpool,
            tc.tile_pool(name="ppool", bufs=1) as ppool,
            tc.tile_pool(name="s2pool", bufs=2) as s2pool,
            tc.tile_pool(name="psum_s", bufs=1, space="PSUM") as psum_s,
            tc.tile_pool(name="psum_a", bufs=4, space="PSUM") as psum_a,
        ):
            ident = consts.tile([P, P], f32)
            make_identity(nc, ident)

            wq_sb = consts.tile([D, D], f32)   # [e, f]
            nc.sync.dma_start(out=wq_sb, in_=wq[:, :])
            wk_sb = consts.tile([D, D], f32)   # [e, d]
            nc.sync.dma_start(out=wk_sb, in_=wk[:, :])
            tgt_sb = consts.tile([P, D], f32)  # [r, f]
            nc.sync.dma_start(out=tgt_sb, in_=tgt[:, :])
            bq_sb = consts.tile([D, 1], f32)
            nc.sync.dma_start(out=bq_sb, in_=bq[:, :])

            # ---- chunk 0 DMA first so the rings start moving immediately
            cts = []
            ct0 = cpool.tile([P, NCH, D], f32, tag="c")
            h = NCH // 2
            nc.sync.dma_start(out=ct0[:, :h, :], in_=cand[:, 0:h, :])
            nc.scalar.dma_start(out=ct0[:, h:, :], in_=cand[:, h:NCH, :])
            cts.append(ct0)

            # u load early on the ACT ring (needed by t1 on ScalarE)
            u_sb = gum.tile([P, N], f32, tag="g0")
            nc.scalar.dma_start(out=u_sb, in_=u[:, :])

            # ---- Qt = ((target @ Wq.T + bq) @ Wk) / sqrt(D),  layout [r, d]
            tgtT_ps = psum_s.tile([D, P], f32)
            nc.tensor.transpose(tgtT_ps, tgt_sb, ident)   # [f, r]
            tgtT_sb = consts.tile([D, P], f32)
            nc.scalar.copy(tgtT_sb, tgtT_ps)

            wqT_ps = psum_s.tile([D, D], f32)
            nc.tensor.transpose(wqT_ps, wq_sb, ident)     # [f, e]
            wqT_sb = consts.tile([D, D], f32)
            nc.scalar.copy(wqT_sb, wqT_ps)

            qT_ps = psum_s.tile([D, P], f32)              # Q.T = [e, r]
            nc.tensor.matmul(qT_ps, wqT_sb, tgtT_sb, start=True, stop=True)
            qT_sb = consts.tile([D, P], f32)
            nc.vector.tensor_scalar_add(qT_sb, qT_ps, bq_sb)

            qt_ps = psum_s.tile([P, D], f32)              # Qt = [r, d]
            nc.tensor.matmul(qt_ps, qT_sb, wk_sb, start=True, stop=True)
            qt_sb = consts.tile([P, D], f32)
            nc.vector.tensor_scalar_mul(qt_sb, qt_ps,
                                        float(1.0 / np.sqrt(np.float32(D))))

            # f32 diagonal weights diag(qt[:, d]) built in one broadcast TT:
            # diags[r, d, c] = ident[r, c] * qt[r, d]
            diags = consts.tile([P, DPE, P], f32)
            id_ap = ident[:, :]
            qt_ap = qt_sb[:, :DPE]
            nc.vector.tensor_tensor(
                out=diags,
                in0=AP(tensor=id_ap.tensor, offset=id_ap.offset,
                       ap=[id_ap.ap[0], [0, DPE], id_ap.ap[1]]),
                in1=AP(tensor=qt_ap.tensor, offset=qt_ap.offset,
                       ap=[qt_ap.ap[0], qt_ap.ap[1], [0, P]]),
                op=alu.mult)

            eps_sb = small.tile([P, 1], f32)
            nc.vector.memset(eps_sb, EPS)

            # gumbel precompute on ScalarE (overlaps the main loop)
            t1 = gum.tile([P, N], f32, tag="g1")
            nc.scalar.activation(t1, u_sb, act.Ln, bias=eps_sb, scale=1.0)
            t2 = gum.tile([P, N], f32, tag="g0")   # overwrites dead u
            nc.scalar.activation(t2, t1, act.Ln, bias=eps_sb, scale=-1.0)

            # ---- main loop
            S = spool.tile([P, N], f32)
            m16 = small.tile([P, NG], f32)
            pss = []
            dve_done = 0   # chunks whose deferred combine has been emitted

            def emit_deferred(g):
                ns = slice(g * NCH, (g + 1) * NCH)
                s2a, s2b, ps = deferred[g]
                nc.vector.tensor_add(S[:, ns], s2a, ps)
                nc.vector.tensor_add(S[:, ns], S[:, ns], s2b)
                nc.vector.tensor_reduce(out=m16[:, g:g + 1], in_=S[:, ns],
                                        axis=mybir.AxisListType.X, op=alu.max)

            deferred = {}
            for g in range(NG):
                if g > 0:
                    ct = cpool.tile([P, NCH, D], f32, tag="c")
                    nc.sync.dma_start(
                        out=ct[:, :h, :],
                        in_=cand[:, g * NCH:g * NCH + h, :])
                    nc.scalar.dma_start(
                        out=ct[:, h:, :],
                        in_=cand[:, g * NCH + h:(g + 1) * NCH, :])
                    cts.append(ct)
                ct = cts[g]

                # Exp act-table preload: dummy tiny op once, mid-loop
                if g == 4:
                    dummy = small.tile([P, 1], f32)
                    nc.scalar.activation(dummy, eps_sb, act.Exp,
                                         bias=0.0, scale=1.0)

                # TensorE: diag-matmul chain for d < DPE
                ps = psum_a.tile([P, NCH], f32, tag="ps")
                for d in range(DPE):
                    nc.tensor.matmul(ps, diags[:, d, :], ct[:, :, d],
                                     start=(d == 0), stop=(d == DPE - 1))
                pss.append(ps)

                # VectorE: two big mult+reduce slices for d >= DPE
                prod = ppool.tile([P, NCH, W1 if W1 >= W2 else W2],
                                  f32, tag="pa")
                s2a = s2pool.tile([P, NCH], f32, tag="sa")
                nc.vector.tensor_tensor(
                    out=prod[:, :, :W1], in0=ct[:, :, DPE:DPE + W1],
                    in1=bcast_mid(qt_sb[:, DPE:DPE + W1], NCH),
                    op=alu.mult)
                nc.vector.tensor_reduce(
                    out=s2a, in_=prod[:, :, :W1],
                    axis=mybir.AxisListType.X, op=alu.add)
                s2b = s2pool.tile([P, NCH], f32, tag="sb")
                nc.vector.tensor_tensor(
                    out=prod[:, :, :W2], in0=ct[:, :, DPE + W1:D],
                    in1=bcast_mid(qt_sb[:, DPE + W1:D], NCH),
                    op=alu.mult)
                nc.vector.tensor_reduce(
                    out=s2b, in_=prod[:, :, :W2],
                    axis=mybir.AxisListType.X, op=alu.add)
                deferred[g] = (s2a, s2b, pss[g])

                # deferred combine for the previous chunk (PE slack = 1 chunk)
                if g > 0:
                    emit_deferred(g - 1)
            emit_deferred(NG - 1)

            # ---- tail
            negm = small.tile([P, 1], f32)
            nc.vector.tensor_reduce(out=negm, in_=m16,
                                    axis=mybir.AxisListType.X,
                                    op=alu.max, negate=True)
            E = ppool.tile([P, N], f32, tag="pa")    # reuses prod buffer
            sumE = small.tile([P, 1], f32)
            nc.scalar.activation(E, S, act.Exp, bias=negm, scale=1.0,
                                 accum_out=sumE)
            beta = small.tile([P, 1], f32)
            nc.vector.tensor_scalar_mul(
                beta, sumE, float(GAMMA / ((1.0 - GAMMA) * N)))
            val = S                                   # S is dead after EXP
            nc.scalar.activation(val, E, act.Ln, bias=beta, scale=1.0)
            phase = gum.tile([P, N], f32, tag="g1")   # reuses dead t1
            nc.vector.tensor_sub(phase, val, t2)

            v8 = small.tile([P, 8], f32)
            idx = small.tile([P, K], u32)
            for r in range(K // 8):
                nc.vector.max(out=v8, in_=phase)
                nc.vector.max_index(out=idx[:, r * 8:(r + 1) * 8],
                                    in_max=v8, in_values=phase)
                if r < K // 8 - 1:
                    nc.vector.match_replace(out=phase, in_to_replace=v8,
                                            in_values=phase,
                                            imm_value=NEG_INF)

            nc.sync.dma_start(out=out[:, :],
                              in_=idx[:, :].bitcast(i32))

    nc.compile()
    return nc


def _get_nc():
    if "nc" not in _CACHE:
        _CACHE["nc"] = _build()
    return _CACHE["nc"]


def kernel(target_embed, candidate_embeds, Wq, bq, Wk, bk=None, u=None,
           num_neighbors=32, **_unused):
    global LAST_RESULT
    from concourse.bass_utils import run_bass_kernel_spmd

    assert int(num_neighbors) == K

    target = np.ascontiguousarray(np.asarray(target_embed, dtype=np.float32))
    cand = np.ascontiguousarray(np.asarray(candidate_embeds, dtype=np.float32))
    uu = np.ascontiguousarray(np.asarray(u, dtype=np.float32))
    wq_ = np.ascontiguousarray(np.asarray(Wq, dtype=np.float32))
    wk_ = np.ascontiguousarray(np.asarray(Wk, dtype=np.float32))
    bq_ = np.ascontiguousarray(np.asarray(bq, dtype=np.float32).reshape(D, 1))

    B = target.shape[0]
    assert B == P * NCORES and cand.shape == (B, N, D)

    in_maps = []
    for c in range(NCORES):
        rs = slice(c * P, (c + 1) * P)
        in_maps.append({
            "target": target[rs],
            "cand": cand[rs],
            "u": uu[rs],
            "Wq": wq_,
            "Wk": wk_,
            "bq": bq_,
        })

    nc = _get_nc()
    res = run_bass_kernel_spmd(nc, in_maps, core_ids=list(range(NCORES)))
    LAST_RESULT = res
    out = np.concatenate([res.results[c]["out"] for c in range(NCORES)],
                         axis=0)
    return out.astype(np.int32)


# revision 11
# speedup vs baseline: 1.1777x; 1.1193x over previous
"""AdaptiveNodeSampler TRN2 kernel — exact f32, fully-overlapped 3-engine.

Per core (128 rows on SBUF partitions, full N=2048 per row):
  - candidates stream in contiguous f32 chunks [P, 128, 128] across the
    two HWDGE rings (SP + ACT), double buffered.
  - scores S[r,n] = sum_d qt[r,d]*C[r,n,d] split across engines:
      TensorE:  d < DPE   via  S += diag(qt[:,d]) @ C[:,:,d]  (PSUM chain)
      VectorE:  d >= DPE  via  2x (broadcast-mult + segmented reduce)
    PE and DVE are decoupled: the PSUM combine for chunk g happens at the
    START of DVE's block g+1 (deferred-by-one), psum bufs=4, so neither
    engine ever waits on the other inside a chunk window.
  - gumbel t1/t2 precomputed on ScalarE during the loop (u loaded early);
    Exp act-table preloaded via a dummy op mid-loop.
  - per-chunk running max partials m16 -> tail reduce (negm).
  - tail: EXP(S-m, accum sumE) -> beta -> val=Ln(E+beta) (written into S)
    -> phase=val-t2 -> top-32 via 4x (max8 -> max_index -> match_replace).

bk cancels in softmax (per-row constant); scale folded into qt.
"""

import os
import sys

sys.path.insert(0, "/opt/trn_rl_repo")

import numpy as np

P = 128
N = 2048
D = 128
K = 32
NCH = 128            # n-chunk size
DPE = int(os.environ.get("ANS_DPE", "48"))   # d's on TensorE
NCORES = 8
GAMMA = 0.1
EPS = 1e-10
NEG_INF = -1.0e30

_CACHE = {}
LAST_RESULT = None


def _build():
    import concourse.bass as bass
    import concourse.bacc as bacc
    import concourse.tile as tile
    from concourse import mybir
    from concourse.masks import make_identity

    f32 = mybir.dt.float32
    i32 = mybir.dt.int32
    u32 = mybir.dt.uint32
    alu = mybir.AluOpType
    act = mybir.ActivationFunctionType
    AP = bass.AP

    def bcast_mid(ap, n):
        """[P, F] -> [P, n, F] with stride-0 middle dim."""
        return AP(tensor=ap.tensor, offset=ap.offset,
                  ap=[ap.ap[0], [0, n], ap.ap[1]])

    DDV = D - DPE
    W1 = DDV // 2          # first DVE slice width
    W2 = DDV - W1
    NG = N // NCH          # number of chunks

    nc = bacc.Bacc("TRN2", target_bir_lowering=False, debug=False,
                   num_devices=NCORES)

    NG = N // NCH
    tgt = nc.declare_dram_parameter("target", [P, D], f32, isOutput=False)
    # chunk-major: each chunk is one contiguous 8MB DRAM block (the host
    # rearranges); measured 382 GB/s/core vs 274 for row-major.
    cand = nc.declare_dram_parameter("cand", [NG, P, NCH, D], f32,
                                     isOutput=False)
    u = nc.declare_dram_parameter("u", [P, N], f32, isOutput=False)
    wq = nc.declare_dram_parameter("Wq", [D, D], f32, isOutput=False)
    wk = nc.declare_dram_parameter("Wk", [D, D], f32, isOutput=False)
    bq = nc.declare_dram_parameter("bq", [D, 1], f32, isOutput=False)
    out = nc.declare_dram_parameter("out", [P, K], i32, isOutput=True)

    with tile.TileContext(nc) as tc:
        with (
            tc.tile_pool(name="consts", bufs=1) as consts,
            tc.tile_pool(name="small", bufs=1) as small,
            tc.tile_pool(name="gum", bufs=1) as gum,
            tc.tile_pool(name="spool", bufs=1) as spool,
            tc.tile_pool(name="cpool", bufs=2) as cpool,
            tc.tile_pool(name="ppool", bufs=1) as ppool,
            tc.tile_pool(name="s2pool", bufs=2) as s2pool,
            tc.tile_pool(name="psum_s", bufs=1, space="PSUM") as psum_s,
            tc.tile_pool(name="psum_a", bufs=4, space="PSUM") as psum_a,
        ):
            ident = consts.tile([P, P], f32)
            make_identity(nc, ident)

            wq_sb = consts.tile([D, D], f32)   # [e, f]
            nc.sync.dma_start(out=wq_sb, in_=wq[:, :])
            wk_sb = consts.tile([D, D], f32)   # [e, d]
            nc.sync.dma_start(out=wk_sb, in_=wk[:, :])
            tgt_sb = consts.tile([P, D], f32)  # [r, f]
            nc.sync.dma_start(out=tgt_sb, in_=tgt[:, :])
            bq_sb = consts.tile([D, 1], f32)
            nc.sync.dma_start(out=bq_sb, in_=bq[:, :])

            # ---- chunk 0 DMA first, split across both rings so it lands
            # at half the whole-chunk latency (it gates compute start)
            cts = []
            ct0 = cpool.tile([P, NCH, D], f32, tag="c")
            h = NCH // 2
            nc.sync.dma_start(out=ct0[:, :h, :], in_=cand[0, :, :h, :])
            nc.scalar.dma_start(out=ct0[:, h:, :], in_=cand[0, :, h:, :])
            cts.append(ct0)

            # u load early on the ACT ring (needed by t1 on ScalarE)
            u_sb = gum.tile([P, N], f32, tag="g0")
            nc.scalar.dma_start(out=u_sb, in_=u[:, :])

            # ---- Qt = ((target @ Wq.T + bq) @ Wk) / sqrt(D),  layout [r, d]
            tgtT_ps = psum_s.tile([D, P], f32)
            nc.tensor.transpose(tgtT_ps, tgt_sb, ident)   # [f, r]
            tgtT_sb = consts.tile([D, P], f32)
            nc.scalar.copy(tgtT_sb, tgtT_ps)

            wqT_ps = psum_s.tile([D, D], f32)
            nc.tensor.transpose(wqT_ps, wq_sb, ident)     # [f, e]
            wqT_sb = consts.tile([D, D], f32)
            nc.scalar.copy(wqT_sb, wqT_ps)

            qT_ps = psum_s.tile([D, P], f32)              # Q.T = [e, r]
            nc.tensor.matmul(qT_ps, wqT_sb, tgtT_sb, start=True, stop=True)
            qT_sb = consts.tile([D, P], f32)
            nc.vector.tensor_scalar_add(qT_sb, qT_ps, bq_sb)

            qt_ps = psum_s.tile([P, D], f32)              # Qt = [r, d]
            nc.tensor.matmul(qt_ps, qT_sb, wk_sb, start=True, stop=True)
            qt_sb = consts.tile([P, D], f32)
            nc.vector.tensor_scalar_mul(qt_sb, qt_ps,
                                        float(1.0 / np.sqrt(np.float32(D))))

            # f32 diagonal weights diag(qt[:, d]) built in one broadcast TT:
            # diags[r, d, c] = ident[r, c] * qt[r, d]
            diags = consts.tile([P, DPE, P], f32)
            id_ap = ident[:, :]
            qt_ap = qt_sb[:, :DPE]
            nc.vector.tensor_tensor(
                out=diags,
                in0=AP(tensor=id_ap.tensor, offset=id_ap.offset,
                       ap=[id_ap.ap[0], [0, DPE], id_ap.ap[1]]),
                in1=AP(tensor=qt_ap.tensor, offset=qt_ap.offset,
                       ap=[qt_ap.ap[0], qt_ap.ap[1], [0, P]]),
                op=alu.mult)

            eps_sb = small.tile([P, 1], f32)
            nc.vector.memset(eps_sb, EPS)

            # gumbel precompute on ScalarE (overlaps the main loop)
            t1 = gum.tile([P, N], f32, tag="g1")
            nc.scalar.activation(t1, u_sb, act.Ln, bias=eps_sb, scale=1.0)
            t2 = gum.tile([P, N], f32, tag="g0")   # overwrites dead u
            nc.scalar.activation(t2, t1, act.Ln, bias=eps_sb, scale=-1.0)

            # ---- main loop
            S = spool.tile([P, N], f32)
            m16 = small.tile([P, NG], f32)
            pss = []
            dve_done = 0   # chunks whose deferred combine has been emitted

            def emit_deferred(g):
                ns = slice(g * NCH, (g + 1) * NCH)
                s2a, s2b, ps = deferred[g]
                nc.vector.tensor_add(S[:, ns], s2a, ps)
                nc.vector.tensor_add(S[:, ns], S[:, ns], s2b)
                nc.vector.tensor_reduce(out=m16[:, g:g + 1], in_=S[:, ns],
                                        axis=mybir.AxisListType.X, op=alu.max)

            deferred = {}
            for g in range(NG):
                if g > 0:
                    ct = cpool.tile([P, NCH, D], f32, tag="c")
                    eng = nc.sync if g % 2 == 0 else nc.scalar
                    eng.dma_start(out=ct, in_=cand[g, :, :, :])
                    cts.append(ct)
                ct = cts[g]

                # Exp act-table preload: dummy tiny op once, mid-loop
                if g == 4:
                    dummy = small.tile([P, 1], f32)
                    nc.scalar.activation(dummy, eps_sb, act.Exp,
                                         bias=0.0, scale=1.0)

                # TensorE: diag-matmul chain for d < DPE
                ps = psum_a.tile([P, NCH], f32, tag="ps")
                for d in range(DPE):
                    nc.tensor.matmul(ps, diags[:, d, :], ct[:, :, d],
                                     start=(d == 0), stop=(d == DPE - 1))
                pss.append(ps)

                # VectorE: two big mult+reduce slices for d >= DPE
                prod = ppool.tile([P, NCH, W1 if W1 >= W2 else W2],
                                  f32, tag="pa")
                s2a = s2pool.tile([P, NCH], f32, tag="sa")
                nc.vector.tensor_tensor(
                    out=prod[:, :, :W1], in0=ct[:, :, DPE:DPE + W1],
                    in1=bcast_mid(qt_sb[:, DPE:DPE + W1], NCH),
                    op=alu.mult)
                nc.vector.tensor_reduce(
                    out=s2a, in_=prod[:, :, :W1],
                    axis=mybir.AxisListType.X, op=alu.add)
                s2b = s2pool.tile([P, NCH], f32, tag="sb")
                nc.vector.tensor_tensor(
                    out=prod[:, :, :W2], in0=ct[:, :, DPE + W1:D],
                    in1=bcast_mid(qt_sb[:, DPE + W1:D], NCH),
                    op=alu.mult)
                nc.vector.tensor_reduce(
                    out=s2b, in_=prod[:, :, :W2],
                    axis=mybir.AxisListType.X, op=alu.add)
                deferred[g] = (s2a, s2b, pss[g])

                # deferred combine for the previous chunk (PE slack = 1 chunk)
                if g > 0:
                    emit_deferred(g - 1)
            emit_deferred(NG - 1)

            # ---- tail
            negm = small.tile([P, 1], f32)
            nc.vector.tensor_reduce(out=negm, in_=m16,
                                    axis=mybir.AxisListType.X,
                                    op=alu.max, negate=True)
            E = ppool.tile([P, N], f32, tag="pa")    # reuses prod buffer
            sumE = small.tile([P, 1], f32)
            nc.scalar.activation(E, S, act.Exp, bias=negm, scale=1.0,
                                 accum_out=sumE)
            beta = small.tile([P, 1], f32)
            nc.vector.tensor_scalar_mul(
                beta, sumE, float(GAMMA / ((1.0 - GAMMA) * N)))
            val = S                                   # S is dead after EXP
            nc.scalar.activation(val, E, act.Ln, bias=beta, scale=1.0)
            phase = gum.tile([P, N], f32, tag="g1")   # reuses dead t1
            nc.vector.tensor_sub(phase, val, t2)

            v8 = small.tile([P, 8], f32)
            idx = small.tile([P, K], u32)
            for r in range(K // 8):
                nc.vector.max(out=v8, in_=phase)
                nc.vector.max_index(out=idx[:, r * 8:(r + 1) * 8],
                                    in_max=v8, in_values=phase)
                if r < K // 8 - 1:
                    nc.vector.match_replace(out=phase, in_to_replace=v8,
                                            in_values=phase,
                                            imm_value=NEG_INF)

            nc.sync.dma_start(out=out[:, :],
                              in_=idx[:, :].bitcast(i32))

    nc.compile()
    return nc


def _get_nc():
    if "nc" not in _CACHE:
        _CACHE["nc"] = _build()
    return _CACHE["nc"]


def kernel(target_embed, candidate_embeds, Wq, bq, Wk, bk=None, u=None,
           num_neighbors=32, **_unused):
    global LAST_RESULT
    from concourse.bass_utils import run_bass_kernel_spmd

    assert int(num_neighbors) == K

    target = np.ascontiguousarray(np.asarray(target_embed, dtype=np.float32))
    cand = np.asarray(candidate_embeds, dtype=np.float32)
    uu = np.ascontiguousarray(np.asarray(u, dtype=np.float32))
    wq_ = np.ascontiguousarray(np.asarray(Wq, dtype=np.float32))
    wk_ = np.ascontiguousarray(np.asarray(Wk, dtype=np.float32))
    bq_ = np.ascontiguousarray(np.asarray(bq, dtype=np.float32).reshape(D, 1))

    B = target.shape[0]
    assert B == P * NCORES and cand.shape == (B, N, D)

    NG = N // NCH
    in_maps = []
    for c in range(NCORES):
        rs = slice(c * P, (c + 1) * P)
        cand_cm = np.ascontiguousarray(
            cand[rs].reshape(P, NG, NCH, D).transpose(1, 0, 2, 3))
        in_maps.append({
            "target": target[rs],
            "cand": cand_cm,
            "u": uu[rs],
            "Wq": wq_,
            "Wk": wk_,
            "bq": bq_,
        })

    nc = _get_nc()
    res = run_bass_kernel_spmd(nc, in_maps, core_ids=list(range(NCORES)))
    LAST_RESULT = res
    out = np.concatenate([res.results[c]["out"] for c in range(NCORES)],
                         axis=0)
    return out.astype(np.int32)


# revision 15
# speedup vs baseline: 1.2492x; 1.0607x over previous
"""AdaptiveNodeSampler TRN2 kernel — exact f32, fully-overlapped 3-engine.

Per core (128 rows on SBUF partitions, full N=2048 per row):
  - candidates stream in contiguous f32 chunks [P, 128, 128] across the
    two HWDGE rings (SP + ACT), double buffered.
  - scores S[r,n] = sum_d qt[r,d]*C[r,n,d] split across engines:
      TensorE:  d < DPE   via  S += diag(qt[:,d]) @ C[:,:,d]  (PSUM chain)
      VectorE:  d >= DPE  via  2x (broadcast-mult + segmented reduce)
    PE and DVE are decoupled: the PSUM combine for chunk g happens at the
    START of DVE's block g+1 (deferred-by-one), psum bufs=4, so neither
    engine ever waits on the other inside a chunk window.
  - gumbel t1/t2 precomputed on ScalarE during the loop (u loaded early);
    Exp act-table preloaded via a dummy op mid-loop.
  - per-chunk running max partials m16 -> tail reduce (negm).
  - tail: EXP(S-m, accum sumE) -> beta -> val=Ln(E+beta) (written into S)
    -> phase=val-t2 -> top-32 via 4x (max8 -> max_index -> match_replace).

bk cancels in softmax (per-row constant); scale folded into qt.
"""

import os
import sys

sys.path.insert(0, "/opt/trn_rl_repo")

import numpy as np

P = 128
N = 2048
D = 128
K = 32
NCH = 128            # n-chunk size
DPE = int(os.environ.get("ANS_DPE", "50"))   # d's on TensorE
NCORES = 8
GAMMA = 0.1
EPS = 1e-10
NEG_INF = -1.0e30

_CACHE = {}
LAST_RESULT = None


def _build():
    import concourse.bass as bass
    import concourse.bacc as bacc
    import concourse.tile as tile
    from concourse import mybir
    from concourse.masks import make_identity

    f32 = mybir.dt.float32
    i32 = mybir.dt.int32
    u32 = mybir.dt.uint32
    alu = mybir.AluOpType
    act = mybir.ActivationFunctionType
    AP = bass.AP

    def bcast_mid(ap, n):
        """[P, F] -> [P, n, F] with stride-0 middle dim."""
        return AP(tensor=ap.tensor, offset=ap.offset,
                  ap=[ap.ap[0], [0, n], ap.ap[1]])

    DDV = D - DPE
    W1 = DDV // 2          # first DVE slice width
    W2 = DDV - W1
    NG = N // NCH          # number of chunks

    nc = bacc.Bacc("TRN2", target_bir_lowering=False, debug=False,
                   num_devices=NCORES)

    NG = N // NCH
    tgt = nc.declare_dram_parameter("target", [P, D], f32, isOutput=False)
    # chunk-major: each chunk is one contiguous 8MB DRAM block (the host
    # rearranges); measured 382 GB/s/core vs 274 for row-major.
    cand = nc.declare_dram_parameter("cand", [NG, P, NCH, D], f32,
                                     isOutput=False)
    u = nc.declare_dram_parameter("u", [P, N], f32, isOutput=False)
    wq = nc.declare_dram_parameter("Wq", [D, D], f32, isOutput=False)
    wk = nc.declare_dram_parameter("Wk", [D, D], f32, isOutput=False)
    bq = nc.declare_dram_parameter("bq", [D, 1], f32, isOutput=False)
    out = nc.declare_dram_parameter("out", [P, K], i32, isOutput=True)

    with tile.TileContext(nc) as tc:
        with (
            tc.tile_pool(name="consts", bufs=1) as consts,
            tc.tile_pool(name="small", bufs=1) as small,
            tc.tile_pool(name="gum", bufs=1) as gum,
            tc.tile_pool(name="spool", bufs=1) as spool,
            tc.tile_pool(name="cpool", bufs=2) as cpool,
            tc.tile_pool(name="ppool", bufs=1) as ppool,
            tc.tile_pool(name="s2pool", bufs=2) as s2pool,
            tc.tile_pool(name="psum_s", bufs=1, space="PSUM") as psum_s,
            tc.tile_pool(name="psum_a", bufs=4, space="PSUM") as psum_a,
        ):
            ident = consts.tile([P, P], f32)
            make_identity(nc, ident)

            wq_sb = consts.tile([D, D], f32)   # [e, f]
            nc.sync.dma_start(out=wq_sb, in_=wq[:, :])
            wk_sb = consts.tile([D, D], f32)   # [e, d]
            nc.sync.dma_start(out=wk_sb, in_=wk[:, :])
            tgt_sb = consts.tile([P, D], f32)  # [r, f]
            nc.sync.dma_start(out=tgt_sb, in_=tgt[:, :])
            bq_sb = consts.tile([D, 1], f32)
            nc.sync.dma_start(out=bq_sb, in_=bq[:, :])

            # ---- chunk 0 DMA first, split across both rings so it lands
            # at half the whole-chunk latency (it gates compute start)
            cts = []
            ct0 = cpool.tile([P, NCH, D], f32, tag="c")
            h = NCH // 2
            nc.sync.dma_start(out=ct0[:, :h, :], in_=cand[0, :, :h, :])
            nc.scalar.dma_start(out=ct0[:, h:, :], in_=cand[0, :, h:, :])
            cts.append(ct0)

            # u load early on the ACT ring (needed by t1 on ScalarE)
            u_sb = gum.tile([P, N], f32, tag="g0")
            nc.scalar.dma_start(out=u_sb, in_=u[:, :])

            # ---- Qt = ((target @ Wq.T + bq) @ Wk) / sqrt(D),  layout [r, d]
            tgtT_ps = psum_s.tile([D, P], f32)
            nc.tensor.transpose(tgtT_ps, tgt_sb, ident)   # [f, r]
            tgtT_sb = consts.tile([D, P], f32)
            nc.scalar.copy(tgtT_sb, tgtT_ps)

            wqT_ps = psum_s.tile([D, D], f32)
            nc.tensor.transpose(wqT_ps, wq_sb, ident)     # [f, e]
            wqT_sb = consts.tile([D, D], f32)
            nc.scalar.copy(wqT_sb, wqT_ps)

            qT_ps = psum_s.tile([D, P], f32)              # Q.T = [e, r]
            nc.tensor.matmul(qT_ps, wqT_sb, tgtT_sb, start=True, stop=True)
            qT_sb = consts.tile([D, P], f32)
            nc.vector.tensor_scalar_add(qT_sb, qT_ps, bq_sb)

            qt_ps = psum_s.tile([P, D], f32)              # Qt = [r, d]
            nc.tensor.matmul(qt_ps, qT_sb, wk_sb, start=True, stop=True)
            qt_sb = consts.tile([P, D], f32)
            nc.vector.tensor_scalar_mul(qt_sb, qt_ps,
                                        float(1.0 / np.sqrt(np.float32(D))))

            # f32 diagonal weights diag(qt[:, d]) built in one broadcast TT:
            # diags[r, d, c] = ident[r, c] * qt[r, d]
            diags = consts.tile([P, DPE, P], f32)
            id_ap = ident[:, :]
            qt_ap = qt_sb[:, :DPE]
            nc.vector.tensor_tensor(
                out=diags,
                in0=AP(tensor=id_ap.tensor, offset=id_ap.offset,
                       ap=[id_ap.ap[0], [0, DPE], id_ap.ap[1]]),
                in1=AP(tensor=qt_ap.tensor, offset=qt_ap.offset,
                       ap=[qt_ap.ap[0], qt_ap.ap[1], [0, P]]),
                op=alu.mult)

            eps_sb = small.tile([P, 1], f32)
            nc.vector.memset(eps_sb, EPS)

            # gumbel precompute on ScalarE (overlaps the main loop)
            t1 = gum.tile([P, N], f32, tag="g1")
            nc.scalar.activation(t1, u_sb, act.Ln, bias=eps_sb, scale=1.0)
            t2 = gum.tile([P, N], f32, tag="g0")   # overwrites dead u
            nc.scalar.activation(t2, t1, act.Ln, bias=eps_sb, scale=-1.0)

            # ---- main loop
            S = spool.tile([P, N], f32)
            pss = []
            dve_done = 0   # chunks whose deferred combine has been emitted

            def emit_deferred(g):
                ns = slice(g * NCH, (g + 1) * NCH)
                s2a, s2b, ps = deferred[g]
                nc.vector.tensor_add(S[:, ns], s2a, ps)
                nc.vector.tensor_add(S[:, ns], S[:, ns], s2b)

            deferred = {}
            for g in range(NG):
                if g > 0:
                    ct = cpool.tile([P, NCH, D], f32, tag="c")
                    eng = nc.sync if g % 2 == 0 else nc.scalar
                    eng.dma_start(out=ct, in_=cand[g, :, :, :])
                    cts.append(ct)
                ct = cts[g]

                # Exp act-table preload: dummy tiny op once, mid-loop
                if g == 4:
                    dummy = small.tile([P, 1], f32)
                    nc.scalar.activation(dummy, eps_sb, act.Exp,
                                         bias=0.0, scale=1.0)

                # TensorE: diag-matmul chain for d < DPE
                ps = psum_a.tile([P, NCH], f32, tag="ps")
                for d in range(DPE):
                    nc.tensor.matmul(ps, diags[:, d, :], ct[:, :, d],
                                     start=(d == 0), stop=(d == DPE - 1))
                pss.append(ps)

                # VectorE: two big mult+reduce slices for d >= DPE
                prod = ppool.tile([P, NCH, W1 if W1 >= W2 else W2],
                                  f32, tag="pa")
                s2a = s2pool.tile([P, NCH], f32, tag="sa")
                nc.vector.tensor_tensor(
                    out=prod[:, :, :W1], in0=ct[:, :, DPE:DPE + W1],
                    in1=bcast_mid(qt_sb[:, DPE:DPE + W1], NCH),
                    op=alu.mult)
                nc.vector.tensor_reduce(
                    out=s2a, in_=prod[:, :, :W1],
                    axis=mybir.AxisListType.X, op=alu.add)
                s2b = s2pool.tile([P, NCH], f32, tag="sb")
                nc.vector.tensor_tensor(
                    out=prod[:, :, :W2], in0=ct[:, :, DPE + W1:D],
                    in1=bcast_mid(qt_sb[:, DPE + W1:D], NCH),
                    op=alu.mult)
                nc.vector.tensor_reduce(
                    out=s2b, in_=prod[:, :, :W2],
                    axis=mybir.AxisListType.X, op=alu.add)
                deferred[g] = (s2a, s2b, pss[g])

                # deferred combine for the previous chunk (PE slack = 1 chunk)
                if g > 0:
                    emit_deferred(g - 1)
            emit_deferred(NG - 1)

            # ---- tail
            negm = small.tile([P, 1], f32)
            nc.vector.tensor_reduce(out=negm, in_=S,
                                    axis=mybir.AxisListType.X,
                                    op=alu.max, negate=True)
            E = ppool.tile([P, N], f32, tag="pa")    # reuses prod buffer
            sumE = small.tile([P, 1], f32)
            nc.scalar.activation(E, S, act.Exp, bias=negm, scale=1.0,
                                 accum_out=sumE)
            beta = small.tile([P, 1], f32)
            nc.vector.tensor_scalar_mul(
                beta, sumE, float(GAMMA / ((1.0 - GAMMA) * N)))
            val = S                                   # S is dead after EXP
            nc.scalar.activation(val, E, act.Ln, bias=beta, scale=1.0)
            phase = gum.tile([P, N], f32, tag="g1")   # reuses dead t1
            nc.vector.tensor_sub(phase, val, t2)

            v8 = small.tile([P, 8], f32)
            idx = small.tile([P, K], u32)
            for r in range(K // 8):
                nc.vector.max(out=v8, in_=phase)
                nc.vector.max_index(out=idx[:, r * 8:(r + 1) * 8],
                                    in_max=v8, in_values=phase)
                if r < K // 8 - 1:
                    nc.vector.match_replace(out=phase, in_to_replace=v8,
                                            in_values=phase,
                                            imm_value=NEG_INF)

            nc.sync.dma_start(out=out[:, :],
                              in_=idx[:, :].bitcast(i32))

    nc.compile()
    return nc


def _get_nc():
    if "nc" not in _CACHE:
        _CACHE["nc"] = _build()
    return _CACHE["nc"]


def kernel(target_embed, candidate_embeds, Wq, bq, Wk, bk=None, u=None,
           num_neighbors=32, **_unused):
    global LAST_RESULT
    from concourse.bass_utils import run_bass_kernel_spmd

    assert int(num_neighbors) == K

    target = np.ascontiguousarray(np.asarray(target_embed, dtype=np.float32))
    cand = np.asarray(candidate_embeds, dtype=np.float32)
    uu = np.ascontiguousarray(np.asarray(u, dtype=np.float32))
    wq_ = np.ascontiguousarray(np.asarray(Wq, dtype=np.float32))
    wk_ = np.ascontiguousarray(np.asarray(Wk, dtype=np.float32))
    bq_ = np.ascontiguousarray(np.asarray(bq, dtype=np.float32).reshape(D, 1))

    B = target.shape[0]
    assert B == P * NCORES and cand.shape == (B, N, D)

    NG = N // NCH
    in_maps = []
    for c in range(NCORES):
        rs = slice(c * P, (c + 1) * P)
        cand_cm = np.ascontiguousarray(
            cand[rs].reshape(P, NG, NCH, D).transpose(1, 0, 2, 3))
        in_maps.append({
            "target": target[rs],
            "cand": cand_cm,
            "u": uu[rs],
            "Wq": wq_,
            "Wk": wk_,
            "bq": bq_,
        })

    nc = _get_nc()
    res = run_bass_kernel_spmd(nc, in_maps, core_ids=list(range(NCORES)))
    LAST_RESULT = res
    out = np.concatenate([res.results[c]["out"] for c in range(NCORES)],
                         axis=0)
    return out.astype(np.int32)
